# revision 3
# baseline (speedup 1.0000x reference)
"""GAT encoder (3-layer) on 8 Trainium2 NeuronCores — V2.

Bottleneck analysis: SWDGE gather descriptor generation costs ~27ns/desc
(measured), so the baseline's 3 gathers/edge (3.5M descs) ~= 95ms. V2 uses
ONE gather per edge slot and builds everything else on-chip:

  - a_full rows are 512B: [h(128 f16), ls@128, 1.0@129, pad] — the single
    gather brings h + ls + a ones column; the scatter matmul rhs is cols
    0:130 so the PSUM accumulator gets numerator (128), garbage (1), and
    denominator (1) in one matmul per tile.
  - the one-hot scatter matrix is built on DVE: oh = (iota == dstpos) from
    a host-static per-slot dstpos f16 table (255 = pad kills the slot).
  - ld[dst] per edge = reduce(oh * ldrep[w]) where ldrep[w] is ld for the
    window's 128 nodes broadcast to all partitions (built per window with
    two tiny matmuls in the dense phase).
  - self-loops are folded into finish_window analytically (no edge slots).

Slots: edges sorted by (core, window, balanced-src-block); per (w,b) padded
to 128*max-over-cores for an SPMD-uniform schedule. S=290k slots/core/layer
vs 389k, 1 desc each vs 3 -> ~870k descs total vs 3.5M.
"""
import sys

sys.path.insert(0, "/opt/trn_rl_repo")

import numpy as np

import os
os.environ.setdefault("JAX_COMPILATION_CACHE_DIR", "/tmp/jax_cache")

import concourse.bacc as bacc
import concourse.bass as bass
import concourse.mybir as mybir
import concourse.tile as tile

F16 = mybir.dt.float16
F32 = mybir.dt.float32
I16 = mybir.dt.int16
ALU = mybir.AluOpType
ACTF = mybir.ActivationFunctionType
AXL = mybir.AxisListType

P = 128
CORES = 8
NB = 5                # balanced src blocks (rows <= 32768 for int16 idx)
NEG_SLOPE = 0.2
SC_WIN = 5            # windows per super-chunk (= live PSUM accumulators)

LAST_RESULTS = None

N_REAL = 150000
USER_COUNT = 100000
N_LAYERS = 3


# ---------------------------------------------------------------- host layout

def build_plan(edge_index, n_real, n_layers):
    R = ((n_real + CORES * P - 1) // (CORES * P)) * P
    NPAD = R * CORES
    NWC = R // P
    BSR = -(-NPAD // NB)                                    # rows per block
    assert BSR <= 32768

    src = np.asarray(edge_index[0], dtype=np.int64)
    dst = np.asarray(edge_index[1], dtype=np.int64)

    core = dst // R
    wloc = (dst % R) // P
    blk = src // BSR

    key = (core * NWC + wloc) * NB + blk
    cnt = np.bincount(key, minlength=CORES * NWC * NB).reshape(CORES, NWC, NB)
    twb = -(-cnt.max(axis=0) // P)          # [NWC, NB]: tiles per (w, block)

    scs = []
    slot_ofs = 0
    for w0 in range(0, NWC, SC_WIN):
        ws = list(range(w0, min(w0 + SC_WIN, NWC)))
        sc_ofs = slot_ofs
        runs = []
        for b in range(NB):
            tiles = []
            r_ofs = slot_ofs
            for w in ws:
                nt = int(twb[w, b])
                if nt:
                    tiles.append((w, nt, slot_ofs))
                    slot_ofs += nt * P
            if slot_ofs > r_ofs:
                runs.append(dict(block=b, tiles=tiles, ofs=r_ofs,
                                 nslots=slot_ofs - r_ofs))
        scs.append(dict(windows=ws, runs=runs, ofs=sc_ofs, end=slot_ofs))
    S = slot_ofs

    order = np.lexsort((blk, wloc, core))
    srcs, dsts = src[order], dst[order]
    cores_s, wl_s, bl_s = core[order], wloc[order], blk[order]

    base = np.zeros((NWC, NB), dtype=np.int64)
    for sc in scs:
        for run in sc["runs"]:
            for (w, nt, ofs) in run["tiles"]:
                base[w, run["block"]] = ofs
    grp = (cores_s * NWC + wl_s) * NB + bl_s
    gstart = np.zeros(CORES * NWC * NB + 1, dtype=np.int64)
    np.cumsum(np.bincount(grp, minlength=CORES * NWC * NB), out=gstart[1:])
    within = np.arange(len(srcs)) - gstart[grp]
    slot = base[wl_s, bl_s] + within

    src_loc = (srcs - bl_s * BSR).astype(np.int16)
    dpos = ((dsts % R) % P).astype(np.float16)

    src_w = np.zeros((CORES, P, S // 16), np.int16)          # pad -> row 0
    src_w[cores_s, slot % 16, slot // 16] = src_loc
    for g in range(1, 8):
        src_w[:, 16 * g:16 * (g + 1)] = src_w[:, :16]

    dstpos = np.full((CORES, P, S // 128), np.float16(255.0), np.float16)
    dstpos[cores_s, slot % 128, slot // 128] = dpos

    max_rt = max((run["nslots"] // P
                  for sc in scs for run in sc["runs"]), default=1)
    max_nt = int(twb.max())
    return dict(R=R, NPAD=NPAD, NWC=NWC, BSR=BSR, S=S, scs=scs,
                max_rt=max_rt, max_nt=max_nt, src_w=src_w, dstpos=dstpos,
                n_layers=n_layers)


# ------------------------------------------------------------ device program

def build_program(plan, compile_program=True):
    R, NPAD, NWC, BSR, S = (plan[k] for k in ("R", "NPAD", "NWC", "BSR", "S"))
    L = plan["n_layers"]
    scs, max_rt, max_nt = plan["scs"], plan["max_rt"], plan["max_nt"]

    nc = bacc.Bacc("TRN2", target_bir_lowering=False, num_devices=CORES,
                   num_swdge_queues=4)

    x0T_d = nc.dram_tensor("x0T", [P, R], F32, kind="ExternalInput")
    W_d = nc.dram_tensor("W", [L, P, P], F32, kind="ExternalInput")
    WT_d = nc.dram_tensor("WT", [L, P, P], F32, kind="ExternalInput")
    as_d = nc.dram_tensor("a_src", [L, P, 1], F32, kind="ExternalInput")
    ad_d = nc.dram_tensor("a_dst", [L, P, 1], F32, kind="ExternalInput")
    bias_d = nc.dram_tensor("bias_rep", [L, P, P], F32, kind="ExternalInput")
    srcw_d = nc.dram_tensor("src_w", [P, S // 16], I16, kind="ExternalInput")
    dpos_d = nc.dram_tensor("dstpos", [P, S // 128], F16,
                            kind="ExternalInput")
    iota_d = nc.dram_tensor("iota_t", [P, P], F16, kind="ExternalInput")
    out_d = nc.dram_tensor("out_x", [R, P], F32, kind="ExternalOutput")

    with tile.TileContext(nc) as tc:
        with tc.tile_pool(name="cst", bufs=1) as cst, \
             tc.tile_pool(name="gbuf", bufs=3) as gbuf, \
             tc.tile_pool(name="obuf", bufs=2) as obuf, \
             tc.tile_pool(name="wbuf", bufs=3) as wbuf, \
             tc.tile_pool(name="pacc", bufs=SC_WIN, space="PSUM") as pacc, \
             tc.tile_pool(name="paux", bufs=1, space="PSUM") as paux, \
             tc.tile_pool(name="dram", bufs=1, space="DRAM") as dram:

            from concourse.masks import make_identity
            ident32 = cst.tile([P, P], F32)
            make_identity(nc, ident32[:])
            iota_sb = cst.tile([P, P], F16)
            nc.sync.dma_start(iota_sb[:], iota_d[:])
            ones_row = cst.tile([1, P], F32)
            nc.vector.memset(ones_row[:], 1.0)

            waug = []
            bias_sb = []
            for l in range(L):
                wa = cst.tile([P, 130], F32, name=f"waug{l}")
                nc.sync.dma_start(wa[:, 0:P], W_d[l])
                wt = wbuf.tile([P, P], F32, tag="xt")
                nc.sync.dma_start(wt[:], WT_d[l])
                for col, vec_d in ((128, as_d), (129, ad_d)):
                    av = wbuf.tile([P, 1], F32, tag="av")
                    nc.sync.dma_start(av[:], vec_d[l])
                    pm = paux.tile([P, 1], F32, tag="ptr")
                    nc.tensor.matmul(pm[:], lhsT=wt[:], rhs=av[:],
                                     start=True, stop=True)
                    nc.vector.tensor_copy(wa[:, col:col + 1], pm[:])
                waug.append(wa)
                bb = cst.tile([P, P], F32, name=f"bias{l}")
                nc.sync.dma_start(bb[:], bias_d[l])
                bias_sb.append(bb)

            srcw_sb = cst.tile([P, S // 16], I16)
            nc.sync.dma_start(srcw_sb[:], srcw_d[:])
            dpos_sb = cst.tile([P, S // 128], F16)
            nc.sync.dma_start(dpos_sb[:], dpos_d[:])

            # per-window captures for own nodes
            hloc = cst.tile([P, NWC * 132], F16)    # h(128), ls, ld per window
            ldrep = cst.tile([P, NWC * P], F16)     # ld bcast along partitions

            a_slice, a_full = [], []
            for l in range(L):
                a_slice.append(dram.tile([R, 256], F16, name=f"a_slice{l}",
                                         tag=f"a_slice{l}"))
                a_full.append(dram.tile([NPAD, 256], F16, name=f"a_full{l}",
                                        tag=f"a_full{l}", addr_space="Shared"))

            eng_alt = [0]

            def copy_any(dst_ap, src_ap):
                eng_alt[0] ^= 1
                if eng_alt[0]:
                    nc.vector.tensor_copy(dst_ap, src_ap)
                else:
                    nc.scalar.copy(dst_ap, src_ap)

            def dense_window(l, w, xt_ap):
                pd = paux.tile([P, 130], F32, tag="pdense")
                nc.tensor.matmul(pd[:], lhsT=xt_ap, rhs=waug[l][:],
                                 start=True, stop=True)
                stage = wbuf.tile([P, 130], F16, tag="stage")
                copy_any(stage[:, 0:P], pd[:, 0:P])
                nc.vector.tensor_copy(stage[:, 128:129], pd[:, 128:129])
                nc.vector.memset(stage[:, 129:130], 1.0)
                nc.sync.dma_start(a_slice[l][w * P:(w + 1) * P, 0:130],
                                  stage[:])
                # own-node captures: h, ls, ld
                copy_any(hloc[:, 132 * w:132 * w + 128], stage[:, 0:P])
                nc.vector.tensor_copy(hloc[:, 132 * w + 128:132 * w + 130],
                                      pd[:, 128:130])
                # ldrep[w]: ld broadcast down partitions (values along free)
                ldcol = wbuf.tile([P, 1], F32, tag="ldcol")
                nc.vector.tensor_copy(ldcol[:], pd[:, 129:130])
                pt = paux.tile([1, P], F32, tag="ptr")
                nc.tensor.transpose(pt[:], ldcol[:], ident32[:])
                ldrow = wbuf.tile([1, P], F32, tag="ldrow")
                nc.vector.tensor_copy(ldrow[:], pt[:])
                pb = paux.tile([P, P], F32, tag="pbc")
                nc.tensor.matmul(pb[:], lhsT=ones_row[:], rhs=ldrow[:],
                                 start=True, stop=True)
                nc.vector.tensor_copy(ldrep[:, P * w:P * (w + 1)], pb[:])

            def finish_window(l, w, pw):
                hl_h = hloc[:, 132 * w:132 * w + 128]
                hl_ls = hloc[:, 132 * w + 128:132 * w + 129]
                hl_ld = hloc[:, 132 * w + 129:132 * w + 130]
                es = wbuf.tile([P, 1], F32, tag="es")
                nc.vector.tensor_tensor(out=es[:], in0=hl_ls, in1=hl_ld,
                                        op=ALU.add)
                nc.vector.scalar_tensor_tensor(
                    out=es[:], in0=es[:], scalar=NEG_SLOPE, op0=ALU.mult,
                    in1=es[:], op1=ALU.max)
                wself = wbuf.tile([P, 1], F32, tag="wself")
                nc.scalar.activation(wself[:], es[:], ACTF.Exp)
                den = wbuf.tile([P, 1], F32, tag="den")
                nc.vector.tensor_tensor(out=den[:], in0=pw[:, 129:130],
                                        in1=wself[:], op=ALU.add)
                rec = wbuf.tile([P, 1], F32, tag="rec")
                nc.vector.reciprocal(rec[:], den[:])
                xn0 = wbuf.tile([P, P], F32, tag="xn0")
                nc.vector.scalar_tensor_tensor(
                    out=xn0[:], in0=hl_h, scalar=wself[:], op0=ALU.mult,
                    in1=pw[:, 0:P], op1=ALU.add)
                xn = wbuf.tile([P, P], F32, tag="xn")
                nc.vector.scalar_tensor_tensor(
                    out=xn[:], in0=xn0[:], scalar=rec[:], op0=ALU.mult,
                    in1=bias_sb[l][:], op1=ALU.add)
                if l < L - 1:
                    pt = paux.tile([P, P], F32, tag="ptr")
                    nc.tensor.transpose(pt[:], xn[:], ident32[:])
                    xt = wbuf.tile([P, P], F32, tag="xt")
                    copy_any(xt[:], pt[:])
                    dense_window(l + 1, w, xt[:])
                else:
                    nc.sync.dma_start(out_d[w * P:(w + 1) * P, :], xn[:])

            # layer 0 dense from x0
            for w in range(NWC):
                xt = wbuf.tile([P, P], F32, tag="xt")
                nc.sync.dma_start(xt[:], x0T_d[:, w * P:(w + 1) * P])
                dense_window(0, w, xt[:])

            rg = [list(range(CORES))]
            qctr = [0]
            for l in range(L):
                nc.gpsimd.collective_compute(
                    "AllGather", ALU.bypass, replica_groups=rg,
                    ins=[a_slice[l][:].opt()], outs=[a_full[l][:].opt()])
                af = a_full[l]
                for sc in scs:
                    if sc["end"] == sc["ofs"]:
                        continue
                    pws = {}
                    remaining = {}
                    for run in sc["runs"]:
                        for (w, nt, _) in run["tiles"]:
                            remaining[w] = remaining.get(w, 0) + nt
                    win_total = dict(remaining)

                    for run in sc["runs"]:
                        b = run["block"]
                        n = run["nslots"]
                        rt = n // P
                        ofs = run["ofs"]
                        brow0 = b * BSR
                        brows = min(BSR, NPAD - brow0)
                        ge = gbuf.tile([P, max_rt, 256], F16, tag="ge")
                        qctr[0] = (qctr[0] + 1) % 4
                        nc.gpsimd.dma_gather(
                            ge[:, 0:rt, :], af[brow0:brow0 + brows, :],
                            srcw_sb[:, ofs // 16:(ofs + n) // 16], n, n, 256,
                            single_packet=False, queue_num=qctr[0])

                        for (w, nt, tofs) in run["tiles"]:
                            t0 = (tofs - ofs) // P
                            gofs = tofs // P
                            oh = obuf.tile([P, max_nt * P], F16, tag="oh")
                            oh_ap = bass.AP(oh.tensor, oh[:].offset,
                                            [oh[:].ap[0], [P, nt], [1, P]])
                            nc.vector.tensor_tensor(
                                out=oh_ap,
                                in0=bass.AP(iota_sb.tensor, iota_sb[:].offset,
                                            [iota_sb[:].ap[0], [0, nt],
                                             [1, P]]),
                                in1=bass.AP(dpos_sb.tensor,
                                            dpos_sb[:].offset + gofs,
                                            [dpos_sb[:].ap[0], [1, nt],
                                             [0, P]]),
                                op=ALU.is_equal)
                            ldc = obuf.tile([P, max_nt * P], F16, tag="ldc")
                            ldc_ap = bass.AP(ldc.tensor, ldc[:].offset,
                                             [ldc[:].ap[0], [P, nt], [1, P]])
                            nc.vector.tensor_tensor(
                                out=ldc_ap, in0=oh_ap,
                                in1=bass.AP(ldrep.tensor,
                                            ldrep[:].offset + P * w,
                                            [ldrep[:].ap[0], [0, nt],
                                             [1, P]]),
                                op=ALU.mult)
                            ec = wbuf.tile([P, max_nt], F32, tag="ec")
                            nc.vector.tensor_reduce(
                                ec[:, 0:nt], ldc_ap, AXL.X, ALU.add)
                            ls_ap = bass.AP(ge.tensor,
                                            ge[:].offset + t0 * 256 + 128,
                                            [ge[:].ap[0], [256, nt], [1, 1]])
                            nc.vector.tensor_tensor(out=ec[:, 0:nt],
                                                    in0=ec[:, 0:nt],
                                                    in1=ls_ap, op=ALU.add)
                            nc.vector.scalar_tensor_tensor(
                                out=ec[:, 0:nt], in0=ec[:, 0:nt],
                                scalar=NEG_SLOPE, op0=ALU.mult,
                                in1=ec[:, 0:nt], op1=ALU.max)
                            w16 = wbuf.tile([P, max_nt], F16, tag="w16")
                            nc.scalar.activation(w16[:, 0:nt], ec[:, 0:nt],
                                                 ACTF.Exp)
                            ow = obuf.tile([P, max_nt * P], F16, tag="ow")
                            ow_ap = bass.AP(ow.tensor, ow[:].offset,
                                            [ow[:].ap[0], [P, nt], [1, P]])
                            nc.vector.tensor_tensor(
                                out=ow_ap, in0=oh_ap,
                                in1=bass.AP(w16.tensor, w16[:].offset,
                                            [w16[:].ap[0], [1, nt], [0, P]]),
                                op=ALU.mult)

                            if w not in pws:
                                pws[w] = pacc.tile([P, 130], F32, tag="pw",
                                                   name=f"pw_{l}_{w}")
                            pw = pws[w]
                            for t in range(nt):
                                nc.tensor.matmul(
                                    pw[:, 0:130],
                                    lhsT=ow[:, t * P:(t + 1) * P],
                                    rhs=bass.AP(ge.tensor,
                                                ge[:].offset + (t0 + t) * 256,
                                                [ge[:].ap[0], [1, 130]]),
                                    start=(remaining[w] == win_total[w]),
                                    stop=(remaining[w] == 1),
                                    skip_group_check=True)
                                remaining[w] -= 1
                                if remaining[w] == 0:
                                    finish_window(l, w, pw)
                                    del pws[w]
    if compile_program:
        nc.compile()
    return nc


# ------------------------------------------------------------------- kernel

_CACHE = {}


def run_plan(plan, x0, W, a_src, a_dst, bias, n_real):
    global LAST_RESULTS
    R, NPAD = plan["R"], plan["NPAD"]
    L = plan["n_layers"]

    key = (plan["S"], plan["NPAD"],
           tuple(tuple((run["block"], tuple(run["tiles"]))
                       for run in sc["runs"]) for sc in plan["scs"]))
    nc = _CACHE.get(key)
    if nc is None:
        nc = build_program(plan)
        _CACHE[key] = nc

    x0p = np.zeros((NPAD, P), np.float32)
    x0p[:n_real] = x0
    bias_rep = np.ascontiguousarray(
        np.broadcast_to(bias[:, None, :], (L, P, P)))
    WT = np.ascontiguousarray(W.transpose(0, 2, 1))
    a_s = np.ascontiguousarray(a_src[:, :, None])
    a_d = np.ascontiguousarray(a_dst[:, :, None])
    iota = np.tile(np.arange(P, dtype=np.float16), (P, 1))

    in_maps = []
    for c in range(CORES):
        x0T = np.ascontiguousarray(x0p[c * R:(c + 1) * R].T)
        in_maps.append({
            "x0T": x0T, "W": W, "WT": WT, "a_src": a_s, "a_dst": a_d,
            "bias_rep": bias_rep, "iota_t": iota,
            "src_w": plan["src_w"][c], "dstpos": plan["dstpos"][c],
        })

    run_once, time_iters, time_pipelined = make_timed_runner(nc, in_maps)
    results = run_once()

    def assemble(res):
        return np.concatenate([res[c]["out_x"]
                               for c in range(CORES)], axis=0)[:n_real]

    x_out = assemble(results)
    # guard against transient post-reset device corruption: retry, then
    # rebuild the runner (fresh input upload) once if NaNs persist.
    for attempt in range(3):
        if not np.isnan(x_out).any():
            break
        if attempt == 1:
            run_once, time_iters, time_pipelined = make_timed_runner(
                nc, in_maps)
        results = run_once()
        x_out = assemble(results)
    LAST_RESULTS = dict(results=results, time_iters=time_iters,
                        time_pipelined=time_pipelined)
    return x_out


def make_timed_runner(nc, in_maps):
    import time

    import jax
    from jax.sharding import Mesh, PartitionSpec
    from jax.experimental.shard_map import shard_map

    from concourse import bass2jax, mybir as mb
    bass2jax.install_neuronx_cc_hook()

    n_cores = len(in_maps)
    partition_name = (nc.partition_id_tensor.name
                      if nc.partition_id_tensor else None)
    in_names, out_names, out_avals, zero_outs = [], [], [], []
    for alloc in nc.m.functions[0].allocations:
        if not isinstance(alloc, mb.MemoryLocationSet):
            continue
        name = alloc.memorylocations[0].name
        if alloc.kind == "ExternalInput":
            if name != partition_name:
                in_names.append(name)
        elif alloc.kind == "ExternalOutput":
            shape = tuple(alloc.tensor_shape)
            dt = mb.dt.np(alloc.dtype)
            out_names.append(name)
            out_avals.append(jax.core.ShapedArray(shape, dt))
            zero_outs.append(np.zeros(shape, dt))
    n_params = len(in_names)
    all_in = list(in_names) + list(out_names)
    if partition_name is not None:
        all_in.append(partition_name)

    def _body(*args):
        operands = list(args)
        if partition_name is not None:
            operands.append(bass2jax.partition_id_tensor())
        outs = bass2jax._bass_exec_p.bind(
            *operands, out_avals=tuple(out_avals), in_names=tuple(all_in),
            out_names=tuple(out_names),
            lowering_input_output_aliases=(),
            sim_require_finite=False, sim_require_nnan=False, nc=nc)
        return tuple(outs)

    devices = jax.devices()[:n_cores]
    mesh = Mesh(np.asarray(devices), ("core",))
    nin = n_params + len(out_names)
    sharded = jax.jit(shard_map(
        _body, mesh=mesh, in_specs=(PartitionSpec("core"),) * nin,
        out_specs=(PartitionSpec("core"),) * len(out_names),
        check_rep=False), keep_unused=True)

    from jax.sharding import NamedSharding
    sh = NamedSharding(mesh, PartitionSpec("core"))
    concat_in = [jax.device_put(
        np.concatenate([np.asarray(in_maps[c][i]) for c in range(n_cores)],
                       axis=0), sh) for i in in_names]
    concat_zero = [jax.device_put(
        np.zeros((n_cores * z.shape[0], *z.shape[1:]), z.dtype), sh)
        for z in zero_outs]

    def run_once():
        outs = sharded(*concat_in, *concat_zero)
        outs = [np.asarray(o) for o in outs]
        return [{name: outs[i].reshape(n_cores, *out_avals[i].shape)[c]
                 for i, name in enumerate(out_names)}
                for c in range(n_cores)]

    def time_iters(n=5):
        ts = []
        for _ in range(n):
            t0 = time.perf_counter()
            outs = sharded(*concat_in, *concat_zero)
            for o in outs:
                o.block_until_ready()
            ts.append(time.perf_counter() - t0)
        return ts

    def time_pipelined(n=10):
        # submit n executions back-to-back, block once: amortizes the axon
        # round-trip so slope ~= device exec time per iteration.
        outs = sharded(*concat_in, *concat_zero)
        for o in outs:
            o.block_until_ready()
        t0 = time.perf_counter()
        all_outs = []
        for _ in range(n):
            all_outs.append(sharded(*concat_in, *concat_zero))
        for outs in all_outs:
            for o in outs:
                o.block_until_ready()
        return time.perf_counter() - t0

    return run_once, time_iters, time_pipelined


def kernel(edge_index, user, item, user_emb, item_emb, W, a_src, a_dst, bias):
    edge_index = np.asarray(edge_index)
    W = np.asarray(W, dtype=np.float32)
    a_src = np.asarray(a_src, dtype=np.float32)
    a_dst = np.asarray(a_dst, dtype=np.float32)
    bias = np.asarray(bias, dtype=np.float32)
    user = np.asarray(user)
    item = np.asarray(item)
    x0 = np.concatenate([np.asarray(user_emb, dtype=np.float32),
                         np.asarray(item_emb, dtype=np.float32)], axis=0)

    plan = build_plan(edge_index, N_REAL, N_LAYERS)
    x3 = run_plan(plan, x0, W, a_src, a_dst, bias, N_REAL)
    return (np.ascontiguousarray(x3[user]),
            np.ascontiguousarray(x3[USER_COUNT + item]))


# revision 4
# speedup vs baseline: 1.0164x; 1.0164x over previous
"""GAT encoder (3-layer) on 8 Trainium2 NeuronCores.

Bottleneck analysis: SWDGE gather descriptor generation costs ~10ns/desc
per queue (measured; ~3.4ns effective across queues), so the original
3 gathers/edge (3.5M descs) dominated at ~95-118ms. This version uses ONE
gather per edge slot and builds everything else on-chip:

  - a_full rows are 512B: [h(128 f16), ls@128, 1.0@129, pad] — the single
    gather brings h + ls + a ones column; the scatter matmul rhs is cols
    0:130 so the PSUM accumulator gets numerator (128), garbage (1), and
    denominator (1) in one matmul per tile.
  - the one-hot scatter matrix is built on DVE once per run:
    oh = (iota == dstpos) over the whole slot strip, from a host-static
    per-slot dstpos f16 table (255 = pad kills the slot).
  - ld[dst] per edge = one run-level reduce of oh * ldrep[w] (per-window
    multiply), where ldrep[w] broadcasts the window's 128 ld values to all
    partitions (two tiny matmuls in the dense phase).
  - e = lrelu(ld_edge + ls), w = exp(e), and ow = oh * w are all run-level
    strip ops; only the ldrep multiply is per window-group.
  - the AllGather moves compact 130-col rows; per-block expand DMAs spread
    them to the 512B-stride gather table.
  - self-loops are folded into finish_window analytically (no edge slots).

Slots: edges sorted by (core, window, balanced-src-block); per (w,b) padded
to 128*max-over-cores for an SPMD-uniform schedule. S=290k slots/core/layer
vs 389k, 1 desc each vs 3 -> ~870k descs total vs 3.5M.
"""
import sys

sys.path.insert(0, "/opt/trn_rl_repo")

import numpy as np

import os
os.environ.setdefault("JAX_COMPILATION_CACHE_DIR", "/tmp/jax_cache")

import concourse.bacc as bacc
import concourse.bass as bass
import concourse.mybir as mybir
import concourse.tile as tile

F16 = mybir.dt.float16
F32 = mybir.dt.float32
I16 = mybir.dt.int16
ALU = mybir.AluOpType
ACTF = mybir.ActivationFunctionType
AXL = mybir.AxisListType

P = 128
CORES = 8
NB = 5                # balanced src blocks (rows <= 32768 for int16 idx)
NEG_SLOPE = 0.2
SC_WIN = 5            # windows per super-chunk (= live PSUM accumulators)

LAST_RESULTS = None

N_REAL = 150000
USER_COUNT = 100000
N_LAYERS = 3


# ---------------------------------------------------------------- host layout

def build_plan(edge_index, n_real, n_layers):
    R = ((n_real + CORES * P - 1) // (CORES * P)) * P
    NPAD = R * CORES
    NWC = R // P
    BSR = -(-NPAD // NB)                                    # rows per block
    assert BSR <= 32768

    src = np.asarray(edge_index[0], dtype=np.int64)
    dst = np.asarray(edge_index[1], dtype=np.int64)

    core = dst // R
    wloc = (dst % R) // P
    blk = src // BSR

    key = (core * NWC + wloc) * NB + blk
    cnt = np.bincount(key, minlength=CORES * NWC * NB).reshape(CORES, NWC, NB)
    twb = -(-cnt.max(axis=0) // P)          # [NWC, NB]: tiles per (w, block)

    scs = []
    slot_ofs = 0
    for w0 in range(0, NWC, SC_WIN):
        ws = list(range(w0, min(w0 + SC_WIN, NWC)))
        sc_ofs = slot_ofs
        runs = []
        for b in range(NB):
            tiles = []
            r_ofs = slot_ofs
            for w in ws:
                nt = int(twb[w, b])
                if nt:
                    tiles.append((w, nt, slot_ofs))
                    slot_ofs += nt * P
            if slot_ofs > r_ofs:
                runs.append(dict(block=b, tiles=tiles, ofs=r_ofs,
                                 nslots=slot_ofs - r_ofs))
        scs.append(dict(windows=ws, runs=runs, ofs=sc_ofs, end=slot_ofs))
    S = slot_ofs

    order = np.lexsort((blk, wloc, core))
    srcs, dsts = src[order], dst[order]
    cores_s, wl_s, bl_s = core[order], wloc[order], blk[order]

    base = np.zeros((NWC, NB), dtype=np.int64)
    for sc in scs:
        for run in sc["runs"]:
            for (w, nt, ofs) in run["tiles"]:
                base[w, run["block"]] = ofs
    grp = (cores_s * NWC + wl_s) * NB + bl_s
    gstart = np.zeros(CORES * NWC * NB + 1, dtype=np.int64)
    np.cumsum(np.bincount(grp, minlength=CORES * NWC * NB), out=gstart[1:])
    within = np.arange(len(srcs)) - gstart[grp]
    slot = base[wl_s, bl_s] + within

    src_loc = (srcs - bl_s * BSR).astype(np.int16)
    dpos = ((dsts % R) % P).astype(np.float16)

    src_w = np.zeros((CORES, P, S // 16), np.int16)          # pad -> row 0
    src_w[cores_s, slot % 16, slot // 16] = src_loc
    for g in range(1, 8):
        src_w[:, 16 * g:16 * (g + 1)] = src_w[:, :16]

    dstpos = np.full((CORES, P, S // 128), np.float16(255.0), np.float16)
    dstpos[cores_s, slot % 128, slot // 128] = dpos

    max_rt = max((run["nslots"] // P
                  for sc in scs for run in sc["runs"]), default=1)
    max_nt = int(twb.max())
    return dict(R=R, NPAD=NPAD, NWC=NWC, BSR=BSR, S=S, scs=scs,
                max_rt=max_rt, max_nt=max_nt, src_w=src_w, dstpos=dstpos,
                n_layers=n_layers)


# ------------------------------------------------------------ device program

def build_program(plan, compile_program=True):
    R, NPAD, NWC, BSR, S = (plan[k] for k in ("R", "NPAD", "NWC", "BSR", "S"))
    L = plan["n_layers"]
    scs, max_rt, max_nt = plan["scs"], plan["max_rt"], plan["max_nt"]

    nc = bacc.Bacc("TRN2", target_bir_lowering=False, num_devices=CORES,
                   num_swdge_queues=4)

    x0T_d = nc.dram_tensor("x0T", [P, R], F32, kind="ExternalInput")
    W_d = nc.dram_tensor("W", [L, P, P], F32, kind="ExternalInput")
    WT_d = nc.dram_tensor("WT", [L, P, P], F32, kind="ExternalInput")
    as_d = nc.dram_tensor("a_src", [L, P, 1], F32, kind="ExternalInput")
    ad_d = nc.dram_tensor("a_dst", [L, P, 1], F32, kind="ExternalInput")
    bias_d = nc.dram_tensor("bias_rep", [L, P, P], F32, kind="ExternalInput")
    srcw_d = nc.dram_tensor("src_w", [P, S // 16], I16, kind="ExternalInput")
    dpos_d = nc.dram_tensor("dstpos", [P, S // 128], F16,
                            kind="ExternalInput")
    iota_d = nc.dram_tensor("iota_t", [P, P], F16, kind="ExternalInput")
    out_d = nc.dram_tensor("out_x", [R, P], F32, kind="ExternalOutput")

    with tile.TileContext(nc) as tc:
        with tc.tile_pool(name="cst", bufs=1) as cst, \
             tc.tile_pool(name="gbuf", bufs=3) as gbuf, \
             tc.tile_pool(name="obuf", bufs=2) as obuf, \
             tc.tile_pool(name="wbuf", bufs=3) as wbuf, \
             tc.tile_pool(name="pacc", bufs=SC_WIN, space="PSUM") as pacc, \
             tc.tile_pool(name="paux", bufs=1, space="PSUM") as paux, \
             tc.tile_pool(name="dram", bufs=1, space="DRAM") as dram:

            from concourse.masks import make_identity
            ident32 = cst.tile([P, P], F32)
            make_identity(nc, ident32[:])
            iota_sb = cst.tile([P, P], F16)
            nc.sync.dma_start(iota_sb[:], iota_d[:])
            ones_row = cst.tile([1, P], F32)
            nc.vector.memset(ones_row[:], 1.0)

            waug = []
            bias_sb = []
            for l in range(L):
                wa = cst.tile([P, 130], F32, name=f"waug{l}")
                nc.sync.dma_start(wa[:, 0:P], W_d[l])
                wt = wbuf.tile([P, P], F32, tag="xt")
                nc.sync.dma_start(wt[:], WT_d[l])
                for col, vec_d in ((128, as_d), (129, ad_d)):
                    av = wbuf.tile([P, 1], F32, tag="av")
                    nc.sync.dma_start(av[:], vec_d[l])
                    pm = paux.tile([P, 1], F32, tag="ptr")
                    nc.tensor.matmul(pm[:], lhsT=wt[:], rhs=av[:],
                                     start=True, stop=True)
                    nc.vector.tensor_copy(wa[:, col:col + 1], pm[:])
                waug.append(wa)
                bb = cst.tile([P, P], F32, name=f"bias{l}")
                nc.sync.dma_start(bb[:], bias_d[l])
                bias_sb.append(bb)

            srcw_sb = cst.tile([P, S // 16], I16)
            nc.sync.dma_start(srcw_sb[:], srcw_d[:])
            dpos_sb = cst.tile([P, S // 128], F16)
            nc.sync.dma_start(dpos_sb[:], dpos_d[:])

            # per-window captures for own nodes
            hloc = cst.tile([P, NWC * 132], F16)    # h(128), ls, ld per window
            ldrep = cst.tile([P, NWC * P], F16)     # ld bcast along partitions

            # compact 130-col slices through the AllGather (48% less D2D
            # traffic); per-block expand DMAs spread rows to the 512B-stride
            # gather table af, and each block's gather only waits on its own
            # expand.
            a_slice, a_fullc, a_full = [], [], []
            for l in range(L):
                a_slice.append(dram.tile([R, 130], F16, name=f"a_slice{l}",
                                         tag=f"a_slice{l}"))
                a_fullc.append(dram.tile([NPAD, 130], F16,
                                         name=f"a_fullc{l}",
                                         tag=f"a_fullc{l}",
                                         addr_space="Shared"))
                a_full.append(dram.tile([NPAD, 256], F16, name=f"a_full{l}",
                                        tag=f"a_full{l}"))

            eng_alt = [0]

            def copy_any(dst_ap, src_ap):
                eng_alt[0] ^= 1
                if eng_alt[0]:
                    nc.vector.tensor_copy(dst_ap, src_ap)
                else:
                    nc.scalar.copy(dst_ap, src_ap)

            def dense_window(l, w, xt_ap):
                pd = paux.tile([P, 130], F32, tag="pdense")
                nc.tensor.matmul(pd[:], lhsT=xt_ap, rhs=waug[l][:],
                                 start=True, stop=True)
                stage = wbuf.tile([P, 130], F16, tag="stage")
                copy_any(stage[:, 0:P], pd[:, 0:P])
                nc.vector.tensor_copy(stage[:, 128:129], pd[:, 128:129])
                nc.vector.memset(stage[:, 129:130], 1.0)
                nc.sync.dma_start(a_slice[l][w * P:(w + 1) * P, :],
                                  stage[:])
                # own-node captures: h, ls, ld
                copy_any(hloc[:, 132 * w:132 * w + 128], stage[:, 0:P])
                nc.vector.tensor_copy(hloc[:, 132 * w + 128:132 * w + 130],
                                      pd[:, 128:130])
                # ldrep[w]: ld broadcast down partitions (values along free)
                ldcol = wbuf.tile([P, 1], F32, tag="ldcol")
                nc.vector.tensor_copy(ldcol[:], pd[:, 129:130])
                pt = paux.tile([1, P], F32, tag="ptr")
                nc.tensor.transpose(pt[:], ldcol[:], ident32[:])
                ldrow = wbuf.tile([1, P], F32, tag="ldrow")
                nc.vector.tensor_copy(ldrow[:], pt[:])
                pb = paux.tile([P, P], F32, tag="pbc")
                nc.tensor.matmul(pb[:], lhsT=ones_row[:], rhs=ldrow[:],
                                 start=True, stop=True)
                nc.vector.tensor_copy(ldrep[:, P * w:P * (w + 1)], pb[:])

            def finish_window(l, w, pw):
                hl_h = hloc[:, 132 * w:132 * w + 128]
                hl_ls = hloc[:, 132 * w + 128:132 * w + 129]
                hl_ld = hloc[:, 132 * w + 129:132 * w + 130]
                es = wbuf.tile([P, 1], F32, tag="es")
                nc.vector.tensor_tensor(out=es[:], in0=hl_ls, in1=hl_ld,
                                        op=ALU.add)
                nc.vector.scalar_tensor_tensor(
                    out=es[:], in0=es[:], scalar=NEG_SLOPE, op0=ALU.mult,
                    in1=es[:], op1=ALU.max)
                wself = wbuf.tile([P, 1], F32, tag="wself")
                nc.scalar.activation(wself[:], es[:], ACTF.Exp)
                den = wbuf.tile([P, 1], F32, tag="den")
                nc.vector.tensor_tensor(out=den[:], in0=pw[:, 129:130],
                                        in1=wself[:], op=ALU.add)
                rec = wbuf.tile([P, 1], F32, tag="rec")
                nc.vector.reciprocal(rec[:], den[:])
                xn0 = wbuf.tile([P, P], F32, tag="xn0")
                nc.vector.scalar_tensor_tensor(
                    out=xn0[:], in0=hl_h, scalar=wself[:], op0=ALU.mult,
                    in1=pw[:, 0:P], op1=ALU.add)
                xn = wbuf.tile([P, P], F32, tag="xn")
                nc.vector.scalar_tensor_tensor(
                    out=xn[:], in0=xn0[:], scalar=rec[:], op0=ALU.mult,
                    in1=bias_sb[l][:], op1=ALU.add)
                if l < L - 1:
                    pt = paux.tile([P, P], F32, tag="ptr")
                    nc.tensor.transpose(pt[:], xn[:], ident32[:])
                    xt = wbuf.tile([P, P], F32, tag="xt")
                    copy_any(xt[:], pt[:])
                    dense_window(l + 1, w, xt[:])
                else:
                    nc.sync.dma_start(out_d[w * P:(w + 1) * P, :], xn[:])

            # layer 0 dense from x0
            for w in range(NWC):
                xt = wbuf.tile([P, P], F32, tag="xt")
                nc.sync.dma_start(xt[:], x0T_d[:, w * P:(w + 1) * P])
                dense_window(0, w, xt[:])

            rg = [list(range(CORES))]
            qctr = [0]
            for l in range(L):
                nc.gpsimd.collective_compute(
                    "AllGather", ALU.bypass, replica_groups=rg,
                    ins=[a_slice[l][:].opt()], outs=[a_fullc[l][:].opt()])
                for b in range(NB):
                    brow0 = b * BSR
                    brows = min(BSR, NPAD - brow0)
                    nc.sync.dma_start(
                        a_full[l][brow0:brow0 + brows, 0:130],
                        a_fullc[l][brow0:brow0 + brows, :])
                af = a_full[l]
                for sc in scs:
                    if sc["end"] == sc["ofs"]:
                        continue
                    pws = {}
                    remaining = {}
                    for run in sc["runs"]:
                        for (w, nt, _) in run["tiles"]:
                            remaining[w] = remaining.get(w, 0) + nt
                    win_total = dict(remaining)

                    for run in sc["runs"]:
                        b = run["block"]
                        n = run["nslots"]
                        rt = n // P
                        ofs = run["ofs"]
                        brow0 = b * BSR
                        brows = min(BSR, NPAD - brow0)
                        ge = gbuf.tile([P, max_rt, 256], F16, tag="ge")
                        qctr[0] = (qctr[0] + 1) % 4
                        nc.gpsimd.dma_gather(
                            ge[:, 0:rt, :], af[brow0:brow0 + brows, :],
                            srcw_sb[:, ofs // 16:(ofs + n) // 16], n, n, 256,
                            single_packet=False, queue_num=qctr[0])

                        # run-level one-hot: dstpos is contiguous across the
                        # run's tiles, so one is_equal covers every group
                        oh = obuf.tile([P, max_rt * P], F16, tag="oh")
                        oh_run = bass.AP(oh.tensor, oh[:].offset,
                                         [oh[:].ap[0], [P, rt], [1, P]])
                        nc.vector.tensor_tensor(
                            out=oh_run,
                            in0=bass.AP(iota_sb.tensor, iota_sb[:].offset,
                                        [iota_sb[:].ap[0], [0, rt], [1, P]]),
                            in1=bass.AP(dpos_sb.tensor,
                                        dpos_sb[:].offset + ofs // P,
                                        [dpos_sb[:].ap[0], [1, rt], [0, P]]),
                            op=ALU.is_equal)

                        # per-group ldrep multiply into a run-level ldc, then
                        # one reduce for the whole run
                        ldc = obuf.tile([P, max_rt * P], F16, tag="ldc")
                        for (w, nt, tofs) in run["tiles"]:
                            t0 = (tofs - ofs) // P
                            nc.vector.tensor_tensor(
                                out=bass.AP(ldc.tensor,
                                            ldc[:].offset + t0 * P,
                                            [ldc[:].ap[0], [P, nt], [1, P]]),
                                in0=bass.AP(oh.tensor,
                                            oh[:].offset + t0 * P,
                                            [oh[:].ap[0], [P, nt], [1, P]]),
                                in1=bass.AP(ldrep.tensor,
                                            ldrep[:].offset + P * w,
                                            [ldrep[:].ap[0], [0, nt],
                                             [1, P]]),
                                op=ALU.mult)
                        lde = wbuf.tile([P, max_rt], F32, tag="lde")
                        nc.vector.tensor_reduce(
                            lde[:, 0:rt], bass.AP(ldc.tensor, ldc[:].offset,
                                                  [ldc[:].ap[0], [P, rt],
                                                   [1, P]]),
                            AXL.X, ALU.add)

                        # run-level: e = lrelu(lde + ls); w = exp(e); ow
                        ls_ap = bass.AP(ge.tensor, ge[:].offset + 128,
                                        [ge[:].ap[0], [256, rt], [1, 1]])
                        nc.vector.tensor_tensor(out=lde[:, 0:rt],
                                                in0=lde[:, 0:rt],
                                                in1=ls_ap, op=ALU.add)
                        nc.vector.scalar_tensor_tensor(
                            out=lde[:, 0:rt], in0=lde[:, 0:rt],
                            scalar=NEG_SLOPE, op0=ALU.mult,
                            in1=lde[:, 0:rt], op1=ALU.max)
                        w16 = wbuf.tile([P, max_rt], F16, tag="w16")
                        nc.scalar.activation(w16[:, 0:rt], lde[:, 0:rt],
                                             ACTF.Exp)
                        ow = obuf.tile([P, max_rt * P], F16, tag="ow")
                        nc.vector.tensor_tensor(
                            out=bass.AP(ow.tensor, ow[:].offset,
                                        [ow[:].ap[0], [P, rt], [1, P]]),
                            in0=oh_run,
                            in1=bass.AP(w16.tensor, w16[:].offset,
                                        [w16[:].ap[0], [1, rt], [0, P]]),
                            op=ALU.mult)

                        for (w, nt, tofs) in run["tiles"]:
                            t0 = (tofs - ofs) // P
                            if w not in pws:
                                pws[w] = pacc.tile([P, 130], F32, tag="pw",
                                                   name=f"pw_{l}_{w}")
                            pw = pws[w]
                            for t in range(nt):
                                nc.tensor.matmul(
                                    pw[:, 0:130],
                                    lhsT=ow[:, (t0 + t) * P:(t0 + t + 1) * P],
                                    rhs=bass.AP(ge.tensor,
                                                ge[:].offset + (t0 + t) * 256,
                                                [ge[:].ap[0], [1, 130]]),
                                    start=(remaining[w] == win_total[w]),
                                    stop=(remaining[w] == 1),
                                    skip_group_check=True)
                                remaining[w] -= 1
                                if remaining[w] == 0:
                                    finish_window(l, w, pw)
                                    del pws[w]
    if compile_program:
        nc.compile()
    return nc


# ------------------------------------------------------------------- kernel

_CACHE = {}


def run_plan(plan, x0, W, a_src, a_dst, bias, n_real):
    global LAST_RESULTS
    R, NPAD = plan["R"], plan["NPAD"]
    L = plan["n_layers"]

    key = (plan["S"], plan["NPAD"],
           tuple(tuple((run["block"], tuple(run["tiles"]))
                       for run in sc["runs"]) for sc in plan["scs"]))
    nc = _CACHE.get(key)
    if nc is None:
        nc = build_program(plan)
        _CACHE[key] = nc

    x0p = np.zeros((NPAD, P), np.float32)
    x0p[:n_real] = x0
    bias_rep = np.ascontiguousarray(
        np.broadcast_to(bias[:, None, :], (L, P, P)))
    WT = np.ascontiguousarray(W.transpose(0, 2, 1))
    a_s = np.ascontiguousarray(a_src[:, :, None])
    a_d = np.ascontiguousarray(a_dst[:, :, None])
    iota = np.tile(np.arange(P, dtype=np.float16), (P, 1))

    in_maps = []
    for c in range(CORES):
        x0T = np.ascontiguousarray(x0p[c * R:(c + 1) * R].T)
        in_maps.append({
            "x0T": x0T, "W": W, "WT": WT, "a_src": a_s, "a_dst": a_d,
            "bias_rep": bias_rep, "iota_t": iota,
            "src_w": plan["src_w"][c], "dstpos": plan["dstpos"][c],
        })

    run_once, time_iters, time_pipelined = make_timed_runner(nc, in_maps)
    results = run_once()

    def assemble(res):
        return np.concatenate([res[c]["out_x"]
                               for c in range(CORES)], axis=0)[:n_real]

    x_out = assemble(results)
    # guard against transient post-reset device corruption: retry, then
    # rebuild the runner (fresh input upload) once if NaNs persist.
    for attempt in range(3):
        if not np.isnan(x_out).any():
            break
        if attempt == 1:
            run_once, time_iters, time_pipelined = make_timed_runner(
                nc, in_maps)
        results = run_once()
        x_out = assemble(results)
    LAST_RESULTS = dict(results=results, time_iters=time_iters,
                        time_pipelined=time_pipelined)
    return x_out


def make_timed_runner(nc, in_maps):
    import time

    import jax
    from jax.sharding import Mesh, PartitionSpec
    from jax.experimental.shard_map import shard_map

    from concourse import bass2jax, mybir as mb
    bass2jax.install_neuronx_cc_hook()

    n_cores = len(in_maps)
    partition_name = (nc.partition_id_tensor.name
                      if nc.partition_id_tensor else None)
    in_names, out_names, out_avals, zero_outs = [], [], [], []
    for alloc in nc.m.functions[0].allocations:
        if not isinstance(alloc, mb.MemoryLocationSet):
            continue
        name = alloc.memorylocations[0].name
        if alloc.kind == "ExternalInput":
            if name != partition_name:
                in_names.append(name)
        elif alloc.kind == "ExternalOutput":
            shape = tuple(alloc.tensor_shape)
            dt = mb.dt.np(alloc.dtype)
            out_names.append(name)
            out_avals.append(jax.core.ShapedArray(shape, dt))
            zero_outs.append(np.zeros(shape, dt))
    n_params = len(in_names)
    all_in = list(in_names) + list(out_names)
    if partition_name is not None:
        all_in.append(partition_name)

    def _body(*args):
        operands = list(args)
        if partition_name is not None:
            operands.append(bass2jax.partition_id_tensor())
        outs = bass2jax._bass_exec_p.bind(
            *operands, out_avals=tuple(out_avals), in_names=tuple(all_in),
            out_names=tuple(out_names),
            lowering_input_output_aliases=(),
            sim_require_finite=False, sim_require_nnan=False, nc=nc)
        return tuple(outs)

    devices = jax.devices()[:n_cores]
    mesh = Mesh(np.asarray(devices), ("core",))
    nin = n_params + len(out_names)
    sharded = jax.jit(shard_map(
        _body, mesh=mesh, in_specs=(PartitionSpec("core"),) * nin,
        out_specs=(PartitionSpec("core"),) * len(out_names),
        check_rep=False), keep_unused=True)

    from jax.sharding import NamedSharding
    sh = NamedSharding(mesh, PartitionSpec("core"))
    concat_in = [jax.device_put(
        np.concatenate([np.asarray(in_maps[c][i]) for c in range(n_cores)],
                       axis=0), sh) for i in in_names]
    concat_zero = [jax.device_put(
        np.zeros((n_cores * z.shape[0], *z.shape[1:]), z.dtype), sh)
        for z in zero_outs]

    def run_once():
        outs = sharded(*concat_in, *concat_zero)
        outs = [np.asarray(o) for o in outs]
        return [{name: outs[i].reshape(n_cores, *out_avals[i].shape)[c]
                 for i, name in enumerate(out_names)}
                for c in range(n_cores)]

    def time_iters(n=5):
        ts = []
        for _ in range(n):
            t0 = time.perf_counter()
            outs = sharded(*concat_in, *concat_zero)
            for o in outs:
                o.block_until_ready()
            ts.append(time.perf_counter() - t0)
        return ts

    def time_pipelined(n=10):
        # submit n executions back-to-back, block once: amortizes the axon
        # round-trip so slope ~= device exec time per iteration.
        outs = sharded(*concat_in, *concat_zero)
        for o in outs:
            o.block_until_ready()
        t0 = time.perf_counter()
        all_outs = []
        for _ in range(n):
            all_outs.append(sharded(*concat_in, *concat_zero))
        for outs in all_outs:
            for o in outs:
                o.block_until_ready()
        return time.perf_counter() - t0

    return run_once, time_iters, time_pipelined


def kernel(edge_index, user, item, user_emb, item_emb, W, a_src, a_dst, bias):
    edge_index = np.asarray(edge_index)
    W = np.asarray(W, dtype=np.float32)
    a_src = np.asarray(a_src, dtype=np.float32)
    a_dst = np.asarray(a_dst, dtype=np.float32)
    bias = np.asarray(bias, dtype=np.float32)
    user = np.asarray(user)
    item = np.asarray(item)
    x0 = np.concatenate([np.asarray(user_emb, dtype=np.float32),
                         np.asarray(item_emb, dtype=np.float32)], axis=0)

    plan = build_plan(edge_index, N_REAL, N_LAYERS)
    x3 = run_plan(plan, x0, W, a_src, a_dst, bias, N_REAL)
    return (np.ascontiguousarray(x3[user]),
            np.ascontiguousarray(x3[USER_COUNT + item]))
